# revision 36
# baseline (speedup 1.0000x reference)
"""AttentionGate kernel for Trainium2 (8 NeuronCores, pure data parallel).

Reference computation (per pixel p, channels c):
    t[p] = sum_c input_[p,c]*wt[c] + bt
    g[p] = sum_c gating [p,c]*wg[c] + bg
    x[p] = sigmoid(w2 * relu(t[p]+g[p]) + b2)
    out[p,c] = input_[p,c] * x[p]

Memory-bound problem: 3 x 256 MiB of f32 I/O.  Two levers vs the f32
pixel-major version (298 us):

1. bf16 I/O — inputs/outputs cross HBM as bfloat16, halving traffic to
   48 MiB/core (DMA roofline ~141-150 us at ~337-358 GB/s/core).  bf16
   keeps f32's exponent range so no subnormal trouble near the harness'
   1e-6 rel-err denominator floor; end-to-end rel err ~8e-3 < 2e-2.
2. Channel-major layout — the host ships x^T/g^T as [C, pixels] so the
   per-pixel 512-channel dot runs on the otherwise-idle PE array as 4
   accumulating [128,128]x[128,512] matmuls per 512-pixel block (the
   DVE scalar_tensor_tensor path used before is capped at 1 elem/cycle
   and was the 155 us bottleneck).  The weight vector is replicated
   across all 128 stationary columns, so every matmul lands the same
   dot product on every PSUM partition -- the partition broadcast
   needed by the final per-pixel multiply comes free.  ScalarE applies
   relu / sigmoid (PSUM->SBUF), and DVE does the final gating multiply
   as plain tensor_tensor mult at 2x bf16 throughput.

Per 512-pixel block: PE ~1.2us, ScalarE ~1.3us, DVE ~0.7us vs a 2.2us
DMA budget -> DMA-bound.  Loads ride the SP HWDGE ring, stores the ACT
ring, so stores never head-of-line block input prefetch.  Uniform 1 MiB
chunk DMAs (4 KB descriptors) with 6-deep input prefetch keep all 16
SDMA engines saturated; measured ~141 us on a quiet machine (~377 GB/s
per core vs the ~358 GB/s nominal HBM-per-NC limit, 48 MiB moved).

Sharding: batch dim 16 -> 2 batches per core, weights replicated.
"""

import sys

import numpy as np

for _p in ("/opt/trn_rl_repo", "/opt/trn_rl_repo/concourse"):
    if _p not in sys.path:
        sys.path.append(_p)

B, H, W, C = 16, 128, 128, 256
NCORES = 8
ROWS = (B // NCORES) * H * W          # pixels per core = 32768
P = 128                                # partitions
A = C // P                             # channel chunks per tensor = 2
BLK = 512                              # pixels per PSUM block (max moving free)
CHUNK = 2048                           # pixels per load/store DMA chunk
# Uniform chunks: 1 MiB loads engage all 16 SDMA engines immediately
# (tapered sub-MiB head loads only ran on 8 engines); compute has ~2.5x
# slack vs DMA so the later first-matmul start doesn't matter.
CHUNKS = [CHUNK] * 16
assert sum(CHUNKS) == ROWS and all(c % BLK == 0 for c in CHUNKS)

_PATCHED = False


def _apply_compat_patches():
    """Work around two ISA-encoding gaps in this container's neuronxcc walrus:

    1. EVENT_SEMAPHORE_RANGE_CLEAR (emitted by the TileContext teardown's
       sem_clear) fails codegen with "ISA wrong length".  Re-execution is
       safe without it (verified on HW), so skip the clear.
    2. The teardown drain carries one sem-wait per logical processor; this
       walrus rejects >1 sync-wait command on a NO_STRUCT ctrl instruction
       ("Too many sync wait commands").  Split the final clock wait into one
       NOP per processor instead.
    """
    global _PATCHED
    if _PATCHED:
        return
    _PATCHED = True

    import concourse.bass as bass
    import concourse.tile as tile_mod
    from bass_rust import ScopedClock, VectorClock
    from concourse.bass import SemaphoreHandle, compact_to_ranges

    def patched_clear(self, sems):
        if not sems:
            return
        sem_nums = [s.num if isinstance(s, SemaphoreHandle) else s for s in sems]
        for sem_range in compact_to_ranges(sem_nums):
            assert self._state.free_isdisjoint(sem_range)
            self.gpsimd.dma_reset(sem_range)
        self._state.prepend_free_semaphores(sem_nums)
        for poison_set in self._tile_sem_poison_stack:
            poison_set.update(sem_nums)

    bass.Bass.clear_and_free_semaphores = patched_clear

    def patched_drain_and_barrier(self, tick_clock, wait_clock):
        gc = tick_clock.global_clock
        for p in range(len(gc)):
            if gc[p] <= 0:
                continue
            vc = VectorClock()
            vc.require_at_least(p, gc[p])
            di = self.nc.sync.nop(nofuse=True)
            wait_clock.add_sem_waits(di.ins, ScopedClock({None: vc}))
        assert self.sems is not None
        popped = self.nc._tile_sem_poison_stack.pop()
        assert popped is self._sem_poison
        # bookkeeping only: recycle sem ids; no dma_reset (the body issues
        # no SWDGE DMAs) and no second barrier -> shorter kernel tail
        sems = list(self.sems.allocated().values())
        from concourse.bass import SemaphoreHandle
        sem_nums = [s.num if isinstance(s, SemaphoreHandle) else s for s in sems]
        self.nc._state.prepend_free_semaphores(sem_nums)
        for poison_set in self.nc._tile_sem_poison_stack:
            poison_set.update(sem_nums)

    tile_mod.TileContext._drain_and_barrier = patched_drain_and_barrier


def _split_multi_waits(nc):
    """This walrus build only encodes ONE sync-wait command per TPB
    instruction.  Hoist all-but-the-last wait of any instruction onto
    freshly inserted same-engine NoOps placed directly before it."""
    import concourse.mybir as mybir

    for f in nc.m.functions:
        for bb in f.blocks:
            insts = bb.instructions  # live list
            i = 0
            while i < len(insts):
                inst = insts[i]
                si = getattr(inst, "sync_info", None)
                if si is not None and len(si.on_wait) > 1:
                    extra, last = list(si.on_wait[:-1]), si.on_wait[-1]
                    for w in extra:
                        nop = mybir.InstNoOp(
                            name=nc.get_next_instruction_name(),
                            engine=inst.engine,
                            sync_info=mybir.SyncInfo(on_wait=[w], on_update=[]),
                            bass_nofuse=True,
                        )
                        insts.insert(i, nop)
                        i += 1
                    inst.sync_info = mybir.SyncInfo(
                        on_wait=[last], on_update=list(si.on_update)
                    )
                i += 1


def _build_program(bt, bg, w2, b2):
    import concourse.bass as bass
    import concourse.mybir as mybir
    from concourse.tile import TileContext

    nc = bass.Bass()
    f32 = mybir.dt.float32
    bf16 = mybir.dt.bfloat16
    x_d = nc.declare_dram_parameter("x", [C, ROWS], bf16, isOutput=False)
    g_d = nc.declare_dram_parameter("g", [C, ROWS], bf16, isOutput=False)
    w_d = nc.declare_dram_parameter("wbb", [P, 2 * A * P], bf16, isOutput=False)
    o_d = nc.declare_dram_parameter("out", [C, ROWS], bf16, isOutput=True)

    # channel c = a*128 + p -> partition p, chunk a; pixel runs contiguous
    x_v = x_d[:].rearrange("(a p) n -> p a n", p=P)
    g_v = g_d[:].rearrange("(a p) n -> p a n", p=P)
    o_v = o_d[:].rearrange("(a p) n -> p a n", p=P)

    with TileContext(nc) as tc:
        with (
            tc.tile_pool(name="wp", bufs=1) as wp,
            tc.tile_pool(name="xp", bufs=6) as xp,
            tc.tile_pool(name="gp", bufs=6) as gp,
            tc.tile_pool(name="op", bufs=5) as op,
            tc.tile_pool(name="sm", bufs=8) as sm,
            tc.psum_pool(name="ps", bufs=8) as psp,
        ):
            # wbb[:, k*128:(k+1)*128] = weight chunk k replicated across all
            # 128 stationary columns, so each accumulating matvec lands the
            # same dot product on every PSUM partition -- the partition
            # broadcast comes free with the matmul.
            wbb = wp.tile([P, 2 * A, P], bf16)
            nc.sync.dma_start(wbb[:], w_d[:])
            relu_bias = wp.tile([P, 1], f32)
            nc.vector.memset(relu_bias[:], float(bt + bg))
            sig_bias = wp.tile([P, 1], f32)
            nc.vector.memset(sig_bias[:], float(b2))

            off = 0
            for cn in CHUNKS:
                span = slice(off, off + cn)
                xt = xp.tile([P, A, CHUNK], bf16, tag="xt")
                nc.sync.dma_start(xt[:, :, 0:cn], x_v[:, :, span])
                gt = gp.tile([P, A, CHUNK], bf16, tag="gt")
                nc.sync.dma_start(gt[:, :, 0:cn], g_v[:, :, span])
                obt = op.tile([P, A, CHUNK], bf16, tag="obt")
                for j in range(cn // BLK):
                    js = slice(j * BLK, (j + 1) * BLK)
                    ps = psp.tile([P, BLK], f32, tag="ps")
                    nc.tensor.matmul(
                        ps[:], wbb[:, 0, :], xt[:, 0, js], start=True, stop=False
                    )
                    nc.tensor.matmul(
                        ps[:], wbb[:, 1, :], xt[:, 1, js], start=False, stop=False
                    )
                    nc.tensor.matmul(
                        ps[:], wbb[:, 2, :], gt[:, 0, js], start=False, stop=False
                    )
                    nc.tensor.matmul(
                        ps[:], wbb[:, 3, :], gt[:, 1, js], start=False, stop=True
                    )
                    s_relu = sm.tile([P, BLK], bf16, tag="srelu")
                    nc.scalar.activation(
                        s_relu[:], ps[:],
                        mybir.ActivationFunctionType.Relu,
                        bias=relu_bias[:],
                    )
                    xsig = sm.tile([P, BLK], bf16, tag="xsig")
                    nc.scalar.activation(
                        xsig[:], s_relu[:],
                        mybir.ActivationFunctionType.Sigmoid,
                        bias=sig_bias[:], scale=float(w2),
                    )
                    nc.vector.tensor_mul(obt[:, 0, js], xt[:, 0, js], xsig[:])
                    nc.vector.tensor_mul(obt[:, 1, js], xt[:, 1, js], xsig[:])
                # out-DMA from the ACT ring: keeps stores off the SP ring so
                # they never head-of-line block input prefetch.
                nc.scalar.dma_start(o_v[:, :, span], obt[:, :, 0:cn])
                off += cn
    _split_multi_waits(nc)
    return nc


def kernel(**inputs):
    _apply_compat_patches()
    import ml_dtypes
    from concourse.bass_utils import run_bass_kernel_spmd

    bf16 = ml_dtypes.bfloat16
    x = np.asarray(inputs["input_"], dtype=np.float32)
    g = np.asarray(inputs["gating_signal"], dtype=np.float32)
    wt = np.asarray(inputs["wt"], dtype=np.float32)
    wg = np.asarray(inputs["wg"], dtype=np.float32)
    bt = float(np.asarray(inputs["bt"]))
    bg = float(np.asarray(inputs["bg"]))
    w2 = float(np.asarray(inputs["w2"]))
    b2 = float(np.asarray(inputs["b2"]))

    nc = _build_program(bt, bg, w2, b2)

    # channel-major per-core views: [ROWS, C] -> [C, ROWS]
    xs = np.ascontiguousarray(
        x.reshape(NCORES, ROWS, C).transpose(0, 2, 1).astype(bf16)
    )
    gs = np.ascontiguousarray(
        g.reshape(NCORES, ROWS, C).transpose(0, 2, 1).astype(bf16)
    )
    # weight chunk k (wt lo/hi, wg lo/hi) replicated across 128 stationary
    # columns: wbb[p, k*128 + q] = chunk_k[p] for all q
    wcols = np.stack([wt[:P], wt[P:], wg[:P], wg[P:]], axis=1)  # [128, 4]
    wbb = np.ascontiguousarray(np.repeat(wcols, P, axis=1).astype(bf16))
    in_maps = [{"x": xs[i], "g": gs[i], "wbb": wbb} for i in range(NCORES)]
    res = run_bass_kernel_spmd(nc, in_maps, list(range(NCORES)))
    out = np.stack([res.results[i]["out"] for i in range(NCORES)], axis=0)
    # [NCORES, C, ROWS] -> [NCORES, ROWS, C] -> [B, H, W, C]
    return (
        out.transpose(0, 2, 1).reshape(B, H, W, C).astype(np.float32)
    )


# revision 37
# speedup vs baseline: 1.0316x; 1.0316x over previous
"""AttentionGate kernel for Trainium2 (8 NeuronCores, pure data parallel).

Reference computation (per pixel p, channels c):
    t[p] = sum_c input_[p,c]*wt[c] + bt
    g[p] = sum_c gating [p,c]*wg[c] + bg
    x[p] = sigmoid(w2 * relu(t[p]+g[p]) + b2)
    out[p,c] = input_[p,c] * x[p]

Memory-bound problem: 3 x 256 MiB of f32 I/O.  Two levers vs the f32
pixel-major version (298 us):

1. bf16 I/O — inputs/outputs cross HBM as bfloat16, halving traffic to
   48 MiB/core (DMA roofline ~141-150 us at ~337-358 GB/s/core).  bf16
   keeps f32's exponent range so no subnormal trouble near the harness'
   1e-6 rel-err denominator floor; end-to-end rel err ~8e-3 < 2e-2.
2. Channel-major layout — the host ships x^T/g^T as [C, pixels] so the
   per-pixel 512-channel dot runs on the otherwise-idle PE array as 4
   accumulating [128,128]x[128,512] matmuls per 512-pixel block (the
   DVE scalar_tensor_tensor path used before is capped at 1 elem/cycle
   and was the 155 us bottleneck).  The weight vector is replicated
   across all 128 stationary columns, so every matmul lands the same
   dot product on every PSUM partition -- the partition broadcast
   needed by the final per-pixel multiply comes free.  ScalarE applies
   relu / sigmoid (PSUM->SBUF), and DVE does the final gating multiply
   as plain tensor_tensor mult at 2x bf16 throughput.

Per 512-pixel block: PE ~1.2us, ScalarE ~1.3us, DVE ~0.7us vs a 2.2us
DMA budget -> DMA-bound.  Loads ride the SP HWDGE ring, stores the ACT
ring, so stores never head-of-line block input prefetch.  Uniform 1 MiB
chunk DMAs (4 KB descriptors) with 6-deep input prefetch keep all 16
SDMA engines saturated; measured ~141 us on a quiet machine (~377 GB/s
per core vs the ~358 GB/s nominal HBM-per-NC limit, 48 MiB moved).

Sharding: batch dim 16 -> 2 batches per core, weights replicated.
"""

import sys

import numpy as np

for _p in ("/opt/trn_rl_repo", "/opt/trn_rl_repo/concourse"):
    if _p not in sys.path:
        sys.path.append(_p)

B, H, W, C = 16, 128, 128, 256
NCORES = 8
ROWS = (B // NCORES) * H * W          # pixels per core = 32768
P = 128                                # partitions
A = C // P                             # channel chunks per tensor = 2
BLK = 512                              # pixels per PSUM block (max moving free)
CHUNK = 2048                           # pixels per load/store DMA chunk
# Uniform chunks: 1 MiB loads engage all 16 SDMA engines immediately
# (tapered sub-MiB head loads only ran on 8 engines); compute has ~2.5x
# slack vs DMA so the later first-matmul start doesn't matter.
CHUNKS = [CHUNK] * 16
assert sum(CHUNKS) == ROWS and all(c % BLK == 0 for c in CHUNKS)

_PATCHED = False


def _apply_compat_patches():
    """Work around two ISA-encoding gaps in this container's neuronxcc walrus:

    1. EVENT_SEMAPHORE_RANGE_CLEAR (emitted by the TileContext teardown's
       sem_clear) fails codegen with "ISA wrong length".  Re-execution is
       safe without it (verified on HW), so skip the clear.
    2. The teardown drain carries one sem-wait per logical processor; this
       walrus rejects >1 sync-wait command on a NO_STRUCT ctrl instruction
       ("Too many sync wait commands").  Split the final clock wait into one
       NOP per processor instead.
    """
    global _PATCHED
    if _PATCHED:
        return
    _PATCHED = True

    import concourse.bass as bass
    import concourse.tile as tile_mod
    from bass_rust import ScopedClock, VectorClock
    from concourse.bass import SemaphoreHandle, compact_to_ranges

    def patched_clear(self, sems):
        if not sems:
            return
        sem_nums = [s.num if isinstance(s, SemaphoreHandle) else s for s in sems]
        for sem_range in compact_to_ranges(sem_nums):
            assert self._state.free_isdisjoint(sem_range)
            self.gpsimd.dma_reset(sem_range)
        self._state.prepend_free_semaphores(sem_nums)
        for poison_set in self._tile_sem_poison_stack:
            poison_set.update(sem_nums)

    bass.Bass.clear_and_free_semaphores = patched_clear

    def patched_drain_and_barrier(self, tick_clock, wait_clock):
        gc = tick_clock.global_clock
        for p in range(len(gc)):
            if gc[p] <= 0:
                continue
            vc = VectorClock()
            vc.require_at_least(p, gc[p])
            di = self.nc.sync.nop(nofuse=True)
            wait_clock.add_sem_waits(di.ins, ScopedClock({None: vc}))
        assert self.sems is not None
        popped = self.nc._tile_sem_poison_stack.pop()
        assert popped is self._sem_poison
        # bookkeeping only: recycle sem ids; no dma_reset (the body issues
        # no SWDGE DMAs) and no second barrier -> shorter kernel tail
        sems = list(self.sems.allocated().values())
        from concourse.bass import SemaphoreHandle
        sem_nums = [s.num if isinstance(s, SemaphoreHandle) else s for s in sems]
        self.nc._state.prepend_free_semaphores(sem_nums)
        for poison_set in self.nc._tile_sem_poison_stack:
            poison_set.update(sem_nums)

    tile_mod.TileContext._drain_and_barrier = patched_drain_and_barrier


def _split_multi_waits(nc):
    """This walrus build only encodes ONE sync-wait command per TPB
    instruction.  Hoist all-but-the-last wait of any instruction onto
    freshly inserted same-engine NoOps placed directly before it."""
    import concourse.mybir as mybir

    for f in nc.m.functions:
        for bb in f.blocks:
            insts = bb.instructions  # live list
            i = 0
            while i < len(insts):
                inst = insts[i]
                si = getattr(inst, "sync_info", None)
                if si is not None and len(si.on_wait) > 1:
                    extra, last = list(si.on_wait[:-1]), si.on_wait[-1]
                    for w in extra:
                        nop = mybir.InstNoOp(
                            name=nc.get_next_instruction_name(),
                            engine=inst.engine,
                            sync_info=mybir.SyncInfo(on_wait=[w], on_update=[]),
                            bass_nofuse=True,
                        )
                        insts.insert(i, nop)
                        i += 1
                    inst.sync_info = mybir.SyncInfo(
                        on_wait=[last], on_update=list(si.on_update)
                    )
                i += 1


def _build_program(bt, bg, w2, b2):
    import concourse.bass as bass
    import concourse.mybir as mybir
    from concourse.tile import TileContext

    nc = bass.Bass()
    f32 = mybir.dt.float32
    bf16 = mybir.dt.bfloat16
    x_d = nc.declare_dram_parameter("x", [C, ROWS], bf16, isOutput=False)
    g_d = nc.declare_dram_parameter("g", [C, ROWS], bf16, isOutput=False)
    w_d = nc.declare_dram_parameter("wbb", [P, 2 * A * P], bf16, isOutput=False)
    o_d = nc.declare_dram_parameter("out", [C, ROWS], bf16, isOutput=True)

    # channel c = a*128 + p -> partition p, chunk a; pixel runs contiguous
    x_v = x_d[:].rearrange("(a p) n -> p a n", p=P)
    g_v = g_d[:].rearrange("(a p) n -> p a n", p=P)
    o_v = o_d[:].rearrange("(a p) n -> p a n", p=P)

    with TileContext(nc) as tc:
        with (
            tc.tile_pool(name="wp", bufs=1) as wp,
            tc.tile_pool(name="xp", bufs=6) as xp,
            tc.tile_pool(name="gp", bufs=6) as gp,
            tc.tile_pool(name="op", bufs=5) as op,
            tc.tile_pool(name="sm", bufs=6) as sm,
            tc.psum_pool(name="ps", bufs=6) as psp,
        ):
            # wbb[:, k*128:(k+1)*128] = weight chunk k replicated across all
            # 128 stationary columns, so each accumulating matvec lands the
            # same dot product on every PSUM partition -- the partition
            # broadcast comes free with the matmul.
            wbb = wp.tile([P, 2 * A, P], bf16)
            nc.sync.dma_start(wbb[:], w_d[:])
            relu_bias = wp.tile([P, 1], f32)
            nc.vector.memset(relu_bias[:], float(bt + bg))
            sig_bias = wp.tile([P, 1], f32)
            nc.vector.memset(sig_bias[:], float(b2))

            off = 0
            for cn in CHUNKS:
                span = slice(off, off + cn)
                xt = xp.tile([P, A, CHUNK], bf16, tag="xt")
                nc.sync.dma_start(xt[:, :, 0:cn], x_v[:, :, span])
                gt = gp.tile([P, A, CHUNK], bf16, tag="gt")
                nc.sync.dma_start(gt[:, :, 0:cn], g_v[:, :, span])
                obt = op.tile([P, A, CHUNK], bf16, tag="obt")
                for j in range(cn // BLK):
                    js = slice(j * BLK, (j + 1) * BLK)
                    ps = psp.tile([P, BLK], f32, tag="ps")
                    nc.tensor.matmul(
                        ps[:], wbb[:, 0, :], xt[:, 0, js], start=True, stop=False
                    )
                    nc.tensor.matmul(
                        ps[:], wbb[:, 1, :], xt[:, 1, js], start=False, stop=False
                    )
                    nc.tensor.matmul(
                        ps[:], wbb[:, 2, :], gt[:, 0, js], start=False, stop=False
                    )
                    nc.tensor.matmul(
                        ps[:], wbb[:, 3, :], gt[:, 1, js], start=False, stop=True
                    )
                    s_relu = sm.tile([P, BLK], bf16, tag="srelu")
                    nc.scalar.activation(
                        s_relu[:], ps[:],
                        mybir.ActivationFunctionType.Relu,
                        bias=relu_bias[:],
                    )
                    xsig = sm.tile([P, BLK], bf16, tag="xsig")
                    nc.scalar.activation(
                        xsig[:], s_relu[:],
                        mybir.ActivationFunctionType.Sigmoid,
                        bias=sig_bias[:], scale=float(w2),
                    )
                    nc.vector.tensor_mul(obt[:, 0, js], xt[:, 0, js], xsig[:])
                    nc.vector.tensor_mul(obt[:, 1, js], xt[:, 1, js], xsig[:])
                # out-DMA from the ACT ring: keeps stores off the SP ring so
                # they never head-of-line block input prefetch.
                nc.scalar.dma_start(o_v[:, :, span], obt[:, :, 0:cn])
                off += cn
    _split_multi_waits(nc)
    return nc


def kernel(**inputs):
    _apply_compat_patches()
    import ml_dtypes
    from concourse.bass_utils import run_bass_kernel_spmd

    bf16 = ml_dtypes.bfloat16
    x = np.asarray(inputs["input_"], dtype=np.float32)
    g = np.asarray(inputs["gating_signal"], dtype=np.float32)
    wt = np.asarray(inputs["wt"], dtype=np.float32)
    wg = np.asarray(inputs["wg"], dtype=np.float32)
    bt = float(np.asarray(inputs["bt"]))
    bg = float(np.asarray(inputs["bg"]))
    w2 = float(np.asarray(inputs["w2"]))
    b2 = float(np.asarray(inputs["b2"]))

    nc = _build_program(bt, bg, w2, b2)

    # channel-major per-core views: [ROWS, C] -> [C, ROWS]
    xs = np.ascontiguousarray(
        x.reshape(NCORES, ROWS, C).transpose(0, 2, 1).astype(bf16)
    )
    gs = np.ascontiguousarray(
        g.reshape(NCORES, ROWS, C).transpose(0, 2, 1).astype(bf16)
    )
    # weight chunk k (wt lo/hi, wg lo/hi) replicated across 128 stationary
    # columns: wbb[p, k*128 + q] = chunk_k[p] for all q
    wcols = np.stack([wt[:P], wt[P:], wg[:P], wg[P:]], axis=1)  # [128, 4]
    wbb = np.ascontiguousarray(np.repeat(wcols, P, axis=1).astype(bf16))
    in_maps = [{"x": xs[i], "g": gs[i], "wbb": wbb} for i in range(NCORES)]
    res = run_bass_kernel_spmd(nc, in_maps, list(range(NCORES)))
    out = np.stack([res.results[i]["out"] for i in range(NCORES)], axis=0)
    # [NCORES, C, ROWS] -> [NCORES, ROWS, C] -> [B, H, W, C]
    return (
        out.transpose(0, 2, 1).reshape(B, H, W, C).astype(np.float32)
    )
